# revision 3
# baseline (speedup 1.0000x reference)
"""Trainium2 Bass kernel for nn_DensityLoss (raw Block mode, SPMD x8, replicated).

Math
----
reference(centers, features, labels) depends only on centers [C=4096, D=256]
(features unused; labels only via N=len(labels)=262144, a constant):

    sq_i  = ||c_i||^2;  m = sum_i c_i;  S = sum sq;  proj_i = c_i . m
    n_i   = C*sq_i + S - 2*proj_i          (center_dist_i = n_i/(C-1); diag==0)
    sum n   = 2*C*S - 2*m.m
    sum n^2 = C^2 q + 3C S^2 + 4 m'Sigma m - 4C (w.m) - 4S (m.m)
        q = sum sq^2, w = sum sq_i c_i, Sigma = X'X
        (sum proj = m.m, sum proj^2 = m'Sigma m)
    result = (sum n) (C-1)^2 / (C * N * (sum n^2 - (sum n)^2/C))

Implementation: per-row sq/256 on DVE (bn_stats, even tiles) and ACT
(Square(x/16)+accum, odd tiles) from the f32 copy; GPSIMD casts X to bf16
on-chip; PE accumulates the Gram of [X | 1 | sq/256] in bf16 (f32 PSUM):
    psA = G[0:128, 0:258]  (Sigma blocks B00/B01 + m0 col 256 + w0 col 257)
    psB = G[128:256,128:258] (B11 + m1 + w1)
S' = sum sq/256 and q' = sum (sq/256)^2 stay in f32 via DVE reductions (they
sit inside the catastrophic var cancellation; bf16 there would be fatal, while
Sigma/m/w only enter small terms - verified ~1e-7 effect).  m'Sigma m via three
[128,128]x[128,1] f32 matvecs on the copied Gram; dot products reduce through
one [128,9] ones-matmul; scalar tail on one partition.  Centers are replicated
to all 8 cores (an 8-core AllReduce has a ~10us floor - more than this whole
kernel).
"""

import numpy as np

C, D = 4096, 256
N_LABELS = 262144
P = 128
NT = C // P            # 32 row tiles
W = D + 2              # 258: [X | ones | sq/256]
CH = 8                 # tiles per DMA chunk
NCHUNK = NT // CH      # 4 chunks of 1 MiB
WP = 264               # padded SBUF row stride (1056 B = 33*32 B, aligned)
N_CORES = 8
DMA_INC = 16           # one +16 per chunk dma_start (per-chunk semaphore)

_CACHE = {}


def _build_nc(repeat=1, tail_repeat=1):
    import concourse.bass as bass
    from concourse import mybir

    f32 = mybir.dt.float32
    bf16 = mybir.dt.bfloat16
    Alu = mybir.AluOpType
    Act = mybir.ActivationFunctionType

    nc = bass.Bass()
    x_ext = nc.declare_dram_parameter("centers", [C, D], f32, isOutput=False)
    out_ext = nc.declare_dram_parameter("out", [1, 1], f32, isOutput=True)

    xv = x_ext[:, :].rearrange("(t p) d -> p t d", p=P)   # [128, 32, 256] view

    from contextlib import ExitStack

    with ExitStack() as ctx:
        en = ctx.enter_context
        xh = en(nc.sbuf_tensor([P, NT, WP], f32))
        xhb = en(nc.sbuf_tensor([P, NT, WP], bf16))
        st6 = en(nc.sbuf_tensor([P, NT // 2, 6], f32))
        mv2 = en(nc.sbuf_tensor([P, NT // 2, 2], f32))
        sq2 = en(nc.sbuf_tensor([P, NT], f32))
        zc = en(nc.sbuf_tensor([P, 1], f32))
        ones_col = en(nc.sbuf_tensor([P, 1], f32))
        Ga = en(nc.sbuf_tensor([P, W], f32))
        Gb = en(nc.sbuf_tensor([P, W - P], f32))
        e = en(nc.sbuf_tensor([P, 9], f32))
        sc = en(nc.sbuf_tensor([1, 32], f32))
        res = en(nc.sbuf_tensor([1, 1], f32))
        psA = en(nc.psum_tensor([P, W], f32))
        psB = en(nc.psum_tensor([P, W - P], f32))
        pv0 = en(nc.psum_tensor([P, 1], f32))
        pt1 = en(nc.psum_tensor([P, 1], f32))
        pv1 = en(nc.psum_tensor([P, 1], f32))
        psS = en(nc.psum_tensor([1, 9], f32))
        scr_a = en(nc.sbuf_tensor([P, NT // 2, D], f32))
        s_dma = [en(nc.semaphore(f"s_dma{i}")) for i in range(NCHUNK)]
        s_pre = en(nc.semaphore("s_pre"))
        s_xb = en(nc.semaphore("s_xb"))
        s_sqa = en(nc.semaphore("s_sqa"))
        s_sqb = en(nc.semaphore("s_sqb"))
        s_mm = en(nc.semaphore("s_mm"))
        s_cpa = en(nc.semaphore("s_cpa"))
        s_cpb = en(nc.semaphore("s_cpb"))
        s_mv = en(nc.semaphore("s_mv"))
        s_e = en(nc.semaphore("s_e"))
        s_sum = en(nc.semaphore("s_sum"))
        s_res = en(nc.semaphore("s_res"))
        s_out = en(nc.semaphore("s_out"))
        block = en(nc.Block())
        m0 = Ga[:, D:D + 1]
        w0 = Ga[:, D + 1:D + 2]
        m1 = Gb[:, D - P:D - P + 1]
        w1 = Gb[:, D - P + 1:D - P + 2]

        @block.sync
        def _(sync):
            for _r in range(repeat):
                for ci in range(NCHUNK):
                    sync.dma_start(
                        out=xh[:, ci * CH:(ci + 1) * CH, 0:D],
                        in_=xv[:, ci * CH:(ci + 1) * CH, :],
                    ).then_inc(s_dma[ci], 16)
            sync.wait_ge(s_res, max(tail_repeat, 1))
            sync.dma_start(out=out_ext[:, :], in_=res[:, :]).then_inc(s_out, 16)
            sync.wait_ge(s_out, 16)

        @block.gpsimd
        def _(gpsimd):
            # on-chip f32 -> bf16 cast of X, chunk by chunk
            for _r in range(repeat):
                for ci in range(NCHUNK):
                    gpsimd.wait_ge(s_dma[ci], DMA_INC * (_r + 1))
                    nc.gpsimd.tensor_copy(
                        out=xhb[:, ci * CH:(ci + 1) * CH, 0:D],
                        in_=xh[:, ci * CH:(ci + 1) * CH, 0:D],
                    ).then_inc(s_xb, 1)

        @block.vector
        def _(vector):
            # preamble constants (cols disjoint from the DMA'd cols 0:256)
            vector.memset(xh[:, :, D:D + 1], 1.0)
            vector.memset(xhb[:, :, D:D + 1], 1.0)
            vector.memset(zc[:, :], 0.0)
            nc.vector.memset(ones_col[:, :], 1.0).then_inc(s_pre, 1)
            # sq/256 for even tiles: bn_stats -> mean^2 + var, in groups of 4
            # with phase-wise drains (DVE pipeline has no intra-engine RAW
            # ordering); after each group, cast the 8 ready sq cols to bf16.
            GR = 4
            for _r in range(repeat):
                for g in range(4):
                    evens = [8 * g + 2 * j for j in range(GR)]
                    for j, t in enumerate(evens):
                        vector.wait_ge(s_dma[t // CH], DMA_INC * (_r + 1))
                        nc.vector.bn_stats(out=st6[:, 4 * g + j, :],
                                           in_=xh[:, t, 0:D])
                    vector.drain()
                    for j in range(GR):
                        nc.vector.bn_aggr(out=mv2[:, 4 * g + j, :],
                                          in_=st6[:, 4 * g + j, :])
                    vector.drain()
                    for j, t in enumerate(evens):
                        nc.vector.tensor_scalar(
                            xh[:, t, D + 1:D + 2],
                            mv2[:, 4 * g + j, 0:1], mv2[:, 4 * g + j, 0:1],
                            mv2[:, 4 * g + j, 1:2],
                            op0=Alu.mult, op1=Alu.add)
                    # odd tiles 8g+1..8g+7 come from ACT
                    vector.wait_ge(s_sqa, (_r * 16) + 4 * (g + 1))
                    vector.drain()
                    nc.vector.tensor_copy(
                        out=xhb[:, 8 * g:8 * g + 8, D + 1:D + 2],
                        in_=xh[:, 8 * g:8 * g + 8, D + 1:D + 2],
                    ).then_inc(s_sqb, 1)
            # S' and q' partials from the f32 sq column (precision-critical)
            sqv = xh[:, :, D + 1]                                  # [128, 32]
            nc.vector.tensor_reduce(e[:, 7:8], sqv, axis=mybir.AxisListType.X,
                                    op=Alu.add)
            nc.vector.tensor_mul(sq2[:, :], sqv, sqv)
            vector.drain()
            nc.vector.tensor_reduce(e[:, 8:9], sq2[:, :],
                                    axis=mybir.AxisListType.X, op=Alu.add)
            # Gram -> SBUF (psB here, psA on ACT in parallel)
            vector.wait_ge(s_mm, 1)
            if tail_repeat == 0:
                nc.vector.memset(res[:, :], 0.0).then_inc(s_res, 1)
            for _t in range(tail_repeat):
              nc.vector.tensor_copy(Gb[:, :], psB[:, :]).then_inc(s_cpb, 1)
              # dot-product columns
              vector.wait_ge(s_cpa, _t + 1)
              vector.drain()
              # copy-only products run in parallel with the PE matvecs
              nc.vector.tensor_mul(e[:, 3:4], m0, m0)
              nc.vector.tensor_mul(e[:, 4:5], m1, m1)
              nc.vector.tensor_mul(e[:, 5:6], w0, m0)
              nc.vector.tensor_mul(e[:, 6:7], w1, m1)
              vector.wait_ge(s_mv, _t + 1)
              nc.vector.tensor_mul(e[:, 0:1], pv0[:, :], m0)
              nc.vector.tensor_mul(e[:, 1:2], pv1[:, :], m1)
              nc.vector.tensor_mul(e[:, 2:3], pt1[:, :], m1).then_inc(s_e, 1)
              vector.wait_ge(s_sum, _t + 1)

              TS = nc.vector.tensor_scalar
              TT = nc.vector.tensor_tensor
              STT = nc.vector.scalar_tensor_tensor

              def s(i):
                  return sc[:, i:i + 1]

              Cf = float(C)
              E = 2.0 ** -20   # pow2 prescale folded into the T-terms
              # levels of independent ops separated by drains (DVE has no
              # intra-engine RAW ordering).  psS: 0 v0m0 | 1 v1m1 | 2 t1m1 |
              # 3 m0m0 | 4 m1m1 | 5 w0m0' | 6 w1m1' | 7 S' | 8 q'
              nc.vector.tensor_copy(sc[:, 0:9], psS[0:1, 0:9])
              vector.drain()
              TT(s(11), s(3), s(4), op=Alu.add)                            # mm
              TT(s(12), s(0), s(1), op=Alu.add)                            # va
              TT(s(14), s(5), s(6), op=Alu.add)                            # w'm
              TT(s(17), s(7), s(7), op=Alu.mult)                           # S'^2
              vector.drain()
              STT(s(13), s(2), 2.0, s(12), op0=Alu.mult, op1=Alu.add)      # mSm
              STT(s(15), s(7), -256.0 * Cf, s(11), op0=Alu.mult,
                  op1=Alu.add)                                             # h=-Sn/2
              TT(s(16), s(7), s(11), op=Alu.mult)                          # S'*mm
              TS(s(20), s(8), Cf * Cf * 65536.0 * E, None, op0=Alu.mult)   # T1
              TS(s(21), s(17), 3.0 * Cf * 65536.0 * E, None, op0=Alu.mult)  # T2
              TS(s(23), s(14), -1024.0 * Cf * E, None, op0=Alu.mult)       # T4
              vector.drain()
              TS(s(22), s(13), 4.0 * E, None, op0=Alu.mult)                # T3
              TS(s(24), s(16), -1024.0 * E, None, op0=Alu.mult)            # T5
              TT(s(18), s(15), s(15), op=Alu.mult)                         # h^2
              vector.drain()
              TS(s(25), s(18), -4.0 / Cf * E, None, op0=Alu.mult)          # T6
              vector.drain()
              nc.vector.tensor_reduce(s(26), sc[:, 20:26],
                                      axis=mybir.AxisListType.X, op=Alu.add)  # d'
              vector.drain()
              nc.vector.reciprocal(s(28), s(26))
              vector.drain()
              k = -2.0 * (Cf - 1.0) ** 2 / (Cf * float(N_LABELS)) * E
              STT(res[:, :], s(15), k, s(28), op0=Alu.mult,
                  op1=Alu.mult).then_inc(s_res, 1)                          # k*h/d'

        @block.scalar
        def _(scalar):
            scalar.wait_ge(s_pre, 1)
            # sq/256 for odd tiles: accum(Square(x/16))
            for _r in range(repeat):
                for j, t in enumerate(range(1, NT, 2)):
                    scalar.wait_ge(s_dma[t // CH], DMA_INC * (_r + 1))
                    nc.scalar.activation(
                        out=scr_a[:, j, :], in_=xh[:, t, 0:D], func=Act.Square,
                        bias=zc[:, :], scale=0.0625,
                        accum_out=xh[:, t, D + 1:D + 2],
                    ).then_inc(s_sqa, 1)
            scalar.wait_ge(s_mm, 1)
            for _t in range(tail_repeat):
                nc.scalar.copy(Ga[:, :], psA[:, :]).then_inc(s_cpa, 1)

        @block.tensor
        def _(tensor):
            tensor.wait_ge(s_pre, 1)
            for _r in range(repeat):
                for t in range(NT):
                    tensor.wait_ge(s_xb, _r * NCHUNK + t // CH + 1)
                    tensor.wait_ge(s_sqb, _r * 4 + t // 8 + 1)
                    first = (_r == 0 and t == 0)
                    last = (_r == repeat - 1 and t == NT - 1)
                    nc.tensor.matmul(psA[:, :], xhb[:, t, 0:P], xhb[:, t, 0:W],
                                     start=first, stop=last)
                    mm = nc.tensor.matmul(psB[:, :], xhb[:, t, P:D],
                                          xhb[:, t, P:W], start=first, stop=last)
                    if last:
                        mm.then_inc(s_mm, 1)
            for _t in range(tail_repeat):
                tensor.wait_ge(s_cpa, _t + 1)
                tensor.wait_ge(s_cpb, _t + 1)
                nc.tensor.matmul(pv0[:, :], Ga[:, 0:P], m0, start=True, stop=True)
                nc.tensor.matmul(pt1[:, :], Ga[:, P:D], m0, start=True, stop=True)
                nc.tensor.matmul(pv1[:, :], Gb[:, 0:P], m1,
                                 start=True, stop=True).then_inc(s_mv, 1)
                tensor.wait_ge(s_e, _t + 1)
                nc.tensor.matmul(psS[:, :], ones_col[:, :], e[:, :],
                                 start=True, stop=True).then_inc(s_sum, 1)

    return nc


def _get_nc(repeat=1, tail_repeat=1):
    key = ("nc", repeat, tail_repeat)
    if key not in _CACHE:
        _CACHE[key] = _build_nc(repeat, tail_repeat)
    return _CACHE[key]


def run(centers: np.ndarray, trace: bool = False):
    """Run the SPMD kernel on cores 0-7; returns (scalar ndarray, results)."""
    from concourse.bass_utils import run_bass_kernel_spmd

    nc = _get_nc()
    x = np.ascontiguousarray(np.asarray(centers, dtype=np.float32))
    in_maps = [{"centers": x} for _ in range(N_CORES)]
    r = run_bass_kernel_spmd(nc, in_maps, core_ids=list(range(N_CORES)),
                             trace=trace)
    # all 8 cores compute the same scalar; median guards a flaky core
    vals = np.array([np.asarray(res["out"]).reshape(()) for res in r.results],
                    dtype=np.float32)
    out = np.median(vals).astype(np.float32).reshape(())
    return out, r


def kernel(centers, features=None, labels=None, **_):
    out, _r = run(centers)
    return out



# revision 8
# speedup vs baseline: 2.7266x; 2.7266x over previous
"""Trainium2 Bass kernel for nn_DensityLoss (raw Block mode, SPMD x8, replicated).

Math
----
reference(centers, features, labels) depends only on centers [C=4096, D=256]
(features unused; labels only via N=len(labels)=262144, a constant):

    sq_i  = ||c_i||^2;  m = sum_i c_i;  S = sum sq;  proj_i = c_i . m
    n_i   = C*sq_i + S - 2*proj_i          (center_dist_i = n_i/(C-1); diag==0)
    sum n   = 2*C*S - 2*m.m
    sum n^2 = C^2 q + 3C S^2 + 4 m'Sigma m - 4C (w.m) - 4S (m.m)
        q = sum sq^2, w = sum sq_i c_i, Sigma = X'X
    result = -2 h (C-1)^2 E / (C * N * d'),  h = m.m - 256 C S',
        d' = E*(sum n^2 - (sum n)^2/C), S' = S/256

Implementation (v2: no GPSIMD -- its f32->bf16 cast was the old kernel's
27us bottleneck; DMA is the 11.1us floor):
  - DMA: p-major contiguous chunks (8 x 4 row-tiles, 512 KiB each).
  - DVE: X cast f32->bf16 (measured ~0.42 cyc/elem), 12 of 32 per-row sq via
    one scalar_tensor_tensor each ((x/256)*x with accum_out = sq/256), the
    per-chunk bf16 cast of the sq' column, and f32 S'/q' partial reductions.
  - ACT: the other 20 sq tiles via activation(Square(x/16), accum_out).
  - PE:  Gram of A=[X|1|sq'] in bf16 as two full-width row blocks
    psA=G[0:128,0:258], psB=G[128:256,0:258] (2x258 cols/tile); tail matvecs
    v0=B00 m0+B01 m1, v1=B10 m0+B11 m1 via Gram symmetry (4 N=1 matmuls),
    cross-partition sums via one f32 ones-matmul psS = 1' @ e[:,0:8].
  - S'/q' stay f32 end-to-end (catastrophic var cancellation; Sigma/m/w only
    enter small terms, bf16 there is ~1e-7 on the result).
Centers replicated to all 8 cores (8-core AllReduce has a ~20us floor; the
whole kernel is shorter).

Timing modes: serial=True chains rounds through s_res (end-to-end latency per
round, the honest exec-time metric); serial=False repeats phase 1 only
(steady-state engine-rate slope).
"""

import numpy as np

C, D = 4096, 256
N_LABELS = 262144
P = 128
NT = C // P            # 32 row tiles
W = D + 2              # 258: [X | ones | sq']
WP = 264               # padded SBUF row stride (1056 B, 32B-aligned)
NCHUNK = 8
CH = NT // NCHUNK      # 4 tiles per chunk
ACT_TILES = [3, 2, 3, 2, 3, 2, 3, 2]   # leading sq tiles per chunk on ACT
N_CORES = 8

_CACHE = {}


def _build_nc(rounds=1, serial=True):
    import concourse.bass as bass
    from concourse import mybir

    f32 = mybir.dt.float32
    bf16 = mybir.dt.bfloat16
    Alu = mybir.AluOpType
    Act = mybir.ActivationFunctionType

    nc = bass.Bass()
    x_ext = nc.declare_dram_parameter("centers", [C, D], f32, isOutput=False)
    out_ext = nc.declare_dram_parameter("out", [1, 1], f32, isOutput=True)

    xv = x_ext[:, :].rearrange("(p t) d -> p t d", p=P)   # [128, 32, 256]

    from contextlib import ExitStack

    with ExitStack() as ctx:
        en = ctx.enter_context
        xh = en(nc.sbuf_tensor([P, NT, WP], f32))
        xhb = en(nc.sbuf_tensor([P, NT, WP], bf16))
        scr = en(nc.sbuf_tensor([P, D], f32))      # ACT Square main-out sink
        scrv = en(nc.sbuf_tensor([P, D], f32))     # DVE STT main-out sink
        sq2 = en(nc.sbuf_tensor([P, NT], f32))
        zc = en(nc.sbuf_tensor([P, 1], f32))
        ones_col = en(nc.sbuf_tensor([P, 1], f32))
        Ga = en(nc.sbuf_tensor([P, W], f32))
        Gb = en(nc.sbuf_tensor([P, W], f32))
        e = en(nc.sbuf_tensor([P, 8], f32))
        sc = en(nc.sbuf_tensor([1, 32], f32))
        res = en(nc.sbuf_tensor([1, 1], f32))
        psA = en(nc.psum_tensor([P, W], f32))
        psB = en(nc.psum_tensor([P, W], f32))
        psV0 = en(nc.psum_tensor([P, 1], f32))
        psV1 = en(nc.psum_tensor([P, 1], f32))
        psS = en(nc.psum_tensor([1, 8], f32))
        s_dma = [en(nc.semaphore(f"s_dma{i}")) for i in range(NCHUNK)]
        s_pre = en(nc.semaphore("s_pre"))
        s_xb = en(nc.semaphore("s_xb"))
        s_sqa = en(nc.semaphore("s_sqa"))
        s_sqd = en(nc.semaphore("s_sqd"))
        s_pe = en(nc.semaphore("s_pe"))
        s_mm = en(nc.semaphore("s_mm"))
        s_cpa = en(nc.semaphore("s_cpa"))
        s_cpb = en(nc.semaphore("s_cpb"))
        s_mv = en(nc.semaphore("s_mv"))
        s_e = en(nc.semaphore("s_e"))
        s_er = en(nc.semaphore("s_er"))
        s_sum = en(nc.semaphore("s_sum"))
        s_res = en(nc.semaphore("s_res"))
        s_out = en(nc.semaphore("s_out"))
        block = en(nc.Block())

        m0 = Ga[:, D:D + 1]
        w0 = Ga[:, D + 1:D + 2]
        m1 = Gb[:, D:D + 1]
        w1 = Gb[:, D + 1:D + 2]
        sqv = xh[:, :, D + 1]                      # [128, 32] sq' column
        n_tail = rounds if serial else 1

        @block.sync
        def _(sync):
            for r in range(rounds):
                for ci in range(NCHUNK):
                    if serial:
                        if r > 0 and ci == 0:
                            sync.wait_ge(s_res, r)
                    else:
                        # WAR: round r overwrites chunk ci once round r-1's
                        # readers (DVE cast+sq'cast, ACT sq, PE) are done
                        if r > 0:
                            sync.wait_ge(s_sqd, NCHUNK * (r - 1) + ci + 1)
                            sync.wait_ge(s_pe, NCHUNK * (r - 1) + ci + 1)
                    sync.dma_start(
                        out=xh[:, ci * CH:(ci + 1) * CH, 0:D],
                        in_=xv[:, ci * CH:(ci + 1) * CH, :],
                    ).then_inc(s_dma[ci], 16)
            sync.wait_ge(s_res, n_tail)
            sync.dma_start(out=out_ext[:, :], in_=res[:, :]).then_inc(s_out, 16)
            sync.wait_ge(s_out, 16)

        @block.vector
        def _(vector):
            # preamble (cols disjoint from DMA'd cols 0:256)
            vector.memset(xhb[:, :, D:D + 1], 1.0)
            vector.memset(zc[:, :], 0.0)
            nc.vector.memset(ones_col[:, :], 1.0).then_inc(s_pre, 1)
            for r in range(rounds):
                for ci in range(NCHUNK):
                    lo, hi = ci * CH, (ci + 1) * CH
                    vector.wait_ge(s_dma[ci], 16 * (r + 1))
                    # X cast for the whole chunk
                    nc.vector.tensor_copy(
                        out=xhb[:, lo:hi, 0:D], in_=xh[:, lo:hi, 0:D],
                    ).then_inc(s_xb, 1)
                    # sq via (x/256)*x with accum -> sq' = sq/256 (f32)
                    for t in range(lo + ACT_TILES[ci], hi):
                        nc.vector.scalar_tensor_tensor(
                            out=scrv[:, :], in0=xh[:, t, 0:D],
                            scalar=1.0 / 256.0, in1=xh[:, t, 0:D],
                            op0=Alu.mult, op1=Alu.mult,
                            accum_out=xh[:, t, D + 1:D + 2])
                    vector.wait_ge(s_sqa, NCHUNK * r + ci + 1)
                    vector.drain()
                    nc.vector.tensor_copy(
                        out=xhb[:, lo:hi, D + 1:D + 2],
                        in_=xh[:, lo:hi, D + 1:D + 2],
                    ).then_inc(s_sqd, 1)
                # S' and q' partials (f32, precision-critical)
                nc.vector.tensor_reduce(e[:, 6:7], sqv,
                                        axis=mybir.AxisListType.X, op=Alu.add)
                nc.vector.tensor_mul(sq2[:, :], sqv, sqv)
                vector.drain()
                nc.vector.tensor_reduce(e[:, 7:8], sq2[:, :],
                                        axis=mybir.AxisListType.X,
                                        op=Alu.add).then_inc(s_er, 1)
                if not serial and r < rounds - 1:
                    continue
                # ---- tail ----
                vector.wait_ge(s_mm, (r + 1) if serial else 1)
                nc.vector.tensor_copy(Gb[:, :], psB[:, :]).then_inc(s_cpb, 1)
                vector.wait_ge(s_mv, (r + 1) if serial else 1)
                nc.vector.tensor_mul(e[:, 0:1], psV0[:, :], m0)
                nc.vector.tensor_mul(e[:, 1:2], psV1[:, :], m1)
                nc.vector.tensor_mul(e[:, 2:3], m0, m0)
                nc.vector.tensor_mul(e[:, 3:4], m1, m1)
                nc.vector.tensor_mul(e[:, 4:5], w0, m0)
                nc.vector.tensor_mul(e[:, 5:6], w1, m1).then_inc(s_e, 1)
                vector.wait_ge(s_sum, (r + 1) if serial else 1)

                TS = nc.vector.tensor_scalar
                TT = nc.vector.tensor_tensor
                STT = nc.vector.scalar_tensor_tensor

                def s(i):
                    return sc[:, i:i + 1]

                Cf = float(C)
                E = 2.0 ** -20
                # psS: 0 v0m0 | 1 v1m1 | 2 m0m0 | 3 m1m1 | 4 w0m0 | 5 w1m1
                #      | 6 S' | 7 q'
                nc.vector.tensor_copy(sc[:, 0:8], psS[0:1, 0:8])
                vector.drain()
                TT(s(11), s(2), s(3), op=Alu.add)                        # mm
                TT(s(12), s(0), s(1), op=Alu.add)                        # mSm
                TT(s(14), s(4), s(5), op=Alu.add)                        # w'm
                TT(s(17), s(6), s(6), op=Alu.mult)                       # S'^2
                vector.drain()
                STT(s(15), s(6), -256.0 * Cf, s(11), op0=Alu.mult,
                    op1=Alu.add)                                         # h
                STT(s(24), s(6), -1024.0 * E, s(11), op0=Alu.mult,
                    op1=Alu.mult)                                        # T5
                TS(s(20), s(7), Cf * Cf * 65536.0 * E, None, op0=Alu.mult)
                TS(s(21), s(17), 3.0 * Cf * 65536.0 * E, None, op0=Alu.mult)
                TS(s(22), s(12), 4.0 * E, None, op0=Alu.mult)            # T3
                TS(s(23), s(14), -1024.0 * Cf * E, None, op0=Alu.mult)   # T4
                vector.drain()
                TT(s(18), s(15), s(15), op=Alu.mult)                     # h^2
                vector.drain()
                TS(s(25), s(18), -4.0 / Cf * E, None, op0=Alu.mult)      # T6
                vector.drain()
                nc.vector.tensor_reduce(s(26), sc[:, 20:26],
                                        axis=mybir.AxisListType.X, op=Alu.add)
                vector.drain()
                nc.vector.reciprocal(s(28), s(26))
                vector.drain()
                k = -2.0 * (Cf - 1.0) ** 2 / (Cf * float(N_LABELS)) * E
                STT(res[:, :], s(15), k, s(28), op0=Alu.mult,
                    op1=Alu.mult).then_inc(s_res, 1)

        @block.scalar
        def _(scalar):
            scalar.wait_ge(s_pre, 1)
            for r in range(rounds):
                if not serial and r > 0:
                    # WAR: round r's accum writes to the sq' col race round
                    # r-1's e6/e7 reductions on DVE
                    scalar.wait_ge(s_er, r)
                for ci in range(NCHUNK):
                    lo = ci * CH
                    scalar.wait_ge(s_dma[ci], 16 * (r + 1))
                    op = None
                    for t in range(lo, lo + ACT_TILES[ci]):
                        op = nc.scalar.activation(
                            out=scr[:, :], in_=xh[:, t, 0:D], func=Act.Square,
                            bias=zc[:, :], scale=0.0625,
                            accum_out=xh[:, t, D + 1:D + 2])
                    op.then_inc(s_sqa, 1)
                if serial or r == rounds - 1:
                    scalar.wait_ge(s_mm, (r + 1) if serial else 1)
                    nc.scalar.copy(Ga[:, :], psA[:, :]).then_inc(s_cpa, 1)

        @block.tensor
        def _(tensor):
            tensor.wait_ge(s_pre, 1)
            for r in range(rounds):
                for t in range(NT):
                    ci = t // CH
                    tensor.wait_ge(s_xb, NCHUNK * r + ci + 1)
                    tensor.wait_ge(s_sqd, NCHUNK * r + ci + 1)
                    first = (t == 0 and (serial or r == 0))
                    last = (t == NT - 1 and (serial or r == rounds - 1))
                    ma = nc.tensor.matmul(psA[:, :], xhb[:, t, 0:P],
                                          xhb[:, t, 0:W], start=first,
                                          stop=last)
                    mm = nc.tensor.matmul(psB[:, :], xhb[:, t, P:D],
                                          xhb[:, t, 0:W], start=first,
                                          stop=last)
                    if t % CH == CH - 1:
                        ma.then_inc(s_pe, 1)
                    if last:
                        mm.then_inc(s_mm, 1)
                if serial or r == rounds - 1:
                    tensor.wait_ge(s_cpa, (r + 1) if serial else 1)
                    tensor.wait_ge(s_cpb, (r + 1) if serial else 1)
                    # v0 = B00 m0 + B01 m1; v1 = B10 m0 + B11 m1 (symmetry)
                    nc.tensor.matmul(psV0[:, :], Ga[:, 0:P], m0,
                                     start=True, stop=False)
                    nc.tensor.matmul(psV0[:, :], Gb[:, 0:P], m1,
                                     start=False, stop=True)
                    nc.tensor.matmul(psV1[:, :], Ga[:, P:D], m0,
                                     start=True, stop=False)
                    nc.tensor.matmul(psV1[:, :], Gb[:, P:D], m1,
                                     start=False, stop=True).then_inc(s_mv, 1)
                    tensor.wait_ge(s_e, (r + 1) if serial else 1)
                    nc.tensor.matmul(psS[:, :], ones_col[:, :], e[:, :],
                                     start=True, stop=True).then_inc(s_sum, 1)

    return nc


def _get_nc(rounds=1, serial=True):
    key = ("nc", rounds, serial)
    if key not in _CACHE:
        _CACHE[key] = _build_nc(rounds, serial)
    return _CACHE[key]


def run(centers: np.ndarray, trace: bool = False):
    """Run the SPMD kernel on cores 0-7; returns (scalar ndarray, results)."""
    from concourse.bass_utils import run_bass_kernel_spmd

    nc = _get_nc()
    x = np.ascontiguousarray(np.asarray(centers, dtype=np.float32))
    in_maps = [{"centers": x} for _ in range(N_CORES)]
    r = run_bass_kernel_spmd(nc, in_maps, core_ids=list(range(N_CORES)),
                             trace=trace)
    # all 8 cores compute the same scalar; median guards a flaky core
    vals = np.array([np.asarray(res["out"]).reshape(()) for res in r.results],
                    dtype=np.float32)
    out = np.median(vals).astype(np.float32).reshape(())
    return out, r


def kernel(centers, features=None, labels=None, **_):
    out, _r = run(centers)
    return out
